# revision 29
# baseline (speedup 1.0000x reference)
"""Trainium2 Bass kernel for nn_DirectionalWedgeBias.

Computes, per (batch b, head h):
    v      = x[b].reshape(T, H, Dh)[:, h, :]          # [T, Dh]
    v_hat  = v / max(||v||_2, eps)  (row-wise)
    S      = A[h] - A[h]^T                            # [Dh, Dh]
    wedge  = (v_hat @ S) @ v_hat^T                    # [T, T]

Full shapes: x [2, 2048, 1024] f32, A [16, 64, 64] f32 -> out [2, 16, 2048, 2048] f32.

Sharding: 32 independent (b, h) pairs split 4-per-core across 8 NeuronCores
(data + head parallel; the tiny skew-symmetric S is replicated/sliced with the
heads). Host pre-slices x into per-core [4, T, Dh] blocks, forms S = A - A^T,
and re-stacks the per-core [4, T, T] results.

Per-core dataflow (Tile framework):
  - load v [2048, 64] as [128 parts, 16, 64]; row-normalize on DVE
    (square+reduce, ACT sqrt, DVE reciprocal, one broadcast multiply)
  - PE-transpose to vT [64, 2048] (Dh on partitions), f32r-rounded
  - SvT [64, 2048] = matmul(lhsT=S, rhs=vT), float32r (1 cyc/row vs 4 for
    fp32; measured rel err ~2e-4 against the fp32 reference)
  - wedge m-tiles: 4 matmuls (N=512, K=64) per [128, 2048] row block; PSUM
    evacuation alternates ScalarE/VectorE; 1 MiB stores alternate between
    the HWDGE (sync) ring and SWDGE (gpsimd) to overlap DMA issue overheads
  - walrus encodes at most ONE semaphore wait on most instructions (and two
    on EventSemaphore), so `_spill_waits` post-processes the Tile-scheduled
    BIR, hoisting excess waits onto preceding same-engine EventSemaphores
    (sequencers run in order, so this is semantics-preserving)

Cost-model (CoreSim) per-core time: ~141 us; HBM write floor for the
64 MiB/core output is ~187 us at ~358 GB/s per core.
"""

import numpy as np

B = 2
T = 2048
D = 1024
H = 16
Dh = 64
N_CORES = 8
PAIRS = (B * H) // N_CORES  # 4 per core
P = 128  # SBUF partitions

_COMPILED = {}

# test-harness knobs (default off; harness calls kernel() with these untouched)
TRACE = False
MM_DTYPE = "float32r"
LAST_RESULT = None


def _build_nc(pairs=PAIRS, t=T, mm_dtype_name="float32r", spill=True, repeat=1):
    _import_concourse()
    from contextlib import ExitStack

    import concourse.bass as bass
    import concourse.tile as tile
    from concourse import mybir

    f32 = mybir.dt.float32
    mmdt = getattr(mybir.dt, mm_dtype_name)
    nt = t // P  # t-tiles per pair
    ng = t // 512  # 512-wide col groups

    def mm_ap(ap):
        return ap.bitcast(mmdt) if mmdt is not f32 else ap

    nc = bass.Bass()
    x_in = nc.declare_dram_parameter("x", [pairs, t, Dh], f32, isOutput=False)
    s_in = nc.declare_dram_parameter("s", [pairs, Dh, Dh], f32, isOutput=False)
    id_in = nc.declare_dram_parameter("ident", [P, P], f32, isOutput=False)
    out_d = nc.declare_dram_parameter("out", [pairs, t, t], f32, isOutput=True)

    with ExitStack() as ctx:
        tc = ctx.enter_context(tile.TileContext(nc))
        const_pool = ctx.enter_context(tc.tile_pool(name="const", bufs=1))
        stage_pool = ctx.enter_context(tc.tile_pool(name="stage", bufs=2))
        pair_pool = ctx.enter_context(tc.tile_pool(name="pair", bufs=2))
        norm_pool = ctx.enter_context(tc.tile_pool(name="norm", bufs=2))
        psw_pool = ctx.enter_context(tc.tile_pool(name="psw", bufs=2, space="PSUM"))
        pst_pool = ctx.enter_context(tc.tile_pool(name="pst", bufs=4, space="PSUM"))
        out_pool = ctx.enter_context(tc.tile_pool(name="outb", bufs=6))

        # identity: DMA-landed, staged through ACT so matmuls only wait on ACT
        id_dma = const_pool.tile([P, P], f32)
        nc.sync.dma_start(out=id_dma, in_=id_in[:, :])
        identity = const_pool.tile([P, P], f32)
        nc.scalar.copy(identity, id_dma)
        # warmup matmul: absorbs the ACT(identity) wait so the first real
        # transpose only needs its DVE wait
        ps_warm = pst_pool.tile([Dh, 512], f32, tag="pst")
        nc.tensor.matmul(
            ps_warm[:1, :1],
            lhsT=identity[:1, :1],
            rhs=identity[:1, :1],
            start=True,
            stop=True,
        )

        for p in [q for _ in range(repeat) for q in range(pairs)]:
            # ---- S (precomputed skew-symmetric), staged through ACT ----
            s_dma = stage_pool.tile([Dh, Dh], f32, tag="sdma")
            nc.scalar.dma_start(out=s_dma, in_=s_in[p])
            s_sb = pair_pool.tile([Dh, Dh], f32, tag="s")
            nc.scalar.copy(mm_ap(s_sb[:]), s_dma)

            # ---- load v as [128, nt, 64], chunked per 512-row group so the
            #      square/reduce work overlaps the remaining loads ----
            v_sb = pair_pool.tile([P, nt, Dh], f32, tag="v")
            vsq = norm_pool.tile([P, nt, Dh], f32, tag="vsq")
            sumsq = norm_pool.tile([P, nt], f32, tag="ss")
            gn = nt // ng  # n-tiles per group (4)
            for g in range(ng):
                nc.scalar.dma_start(
                    out=v_sb[:, g * gn : (g + 1) * gn, :],
                    in_=x_in[p][g * 512 : (g + 1) * 512, :].rearrange(
                        "(n p) d -> p n d", p=P
                    ),
                )
                nc.vector.tensor_mul(
                    vsq[:, g * gn : (g + 1) * gn, :],
                    v_sb[:, g * gn : (g + 1) * gn, :],
                    v_sb[:, g * gn : (g + 1) * gn, :],
                )
                nc.vector.reduce_sum(
                    sumsq[:, g * gn : (g + 1) * gn],
                    vsq[:, g * gn : (g + 1) * gn, :],
                    axis=mybir.AxisListType.X,
                )
            nrm = norm_pool.tile([P, nt], f32, tag="nrm")
            nc.scalar.activation(nrm, sumsq, mybir.ActivationFunctionType.Sqrt)
            rinv = norm_pool.tile([P, nt], f32, tag="rinv")
            nc.vector.reciprocal(rinv, nrm)

            # per group: normalize (fresh DVE-owned tile), PE-transpose,
            # evacuate, and immediately form that group's SvT slice so the
            # first wedge tiles can start before later groups finish
            v_hat = pair_pool.tile([P, nt, Dh], f32, tag="vhat")
            vt_sb = pair_pool.tile([Dh, t], f32, tag="vt")
            svt_sb = pair_pool.tile([Dh, t], f32, tag="svt")
            for g in range(ng):
                rb = (
                    rinv[:, g * gn : (g + 1) * gn]
                    .unsqueeze(-1)
                    .broadcast_to((P, gn, Dh))
                )
                nc.vector.tensor_mul(
                    v_hat[:, g * gn : (g + 1) * gn, :],
                    v_sb[:, g * gn : (g + 1) * gn, :],
                    rb,
                )
                ps_vt = pst_pool.tile([Dh, 512], f32, tag="pst")
                for j in range(gn):
                    n = g * gn + j
                    nc.tensor.transpose(
                        ps_vt[:, j * P : (j + 1) * P], v_hat[:, n, :], identity
                    )
                nc.vector.tensor_copy(mm_ap(vt_sb[:, g * 512 : (g + 1) * 512]), ps_vt)
                ps_sv = pst_pool.tile([Dh, 512], f32, tag="pst")
                nc.tensor.matmul(
                    ps_sv,
                    lhsT=mm_ap(s_sb[:]),
                    rhs=mm_ap(vt_sb[:, g * 512 : (g + 1) * 512]),
                    start=True,
                    stop=True,
                )
                nc.scalar.copy(mm_ap(svt_sb[:, g * 512 : (g + 1) * 512]), ps_sv)

            # ---- wedge tiles: [128, W] halves into a [128, 2W] out tile;
            #      evacuation alternates ACT/DVE; 1 MiB stores alternate
            #      between the HWDGE (sync) ring and SWDGE (gpsimd) ----
            W = 1024 if ng % 2 == 0 else 512
            wq = W // 512
            halves = t // W
            for m in range(nt):
                ob = out_pool.tile([P, t], f32, tag="ob")
                for h in range(halves):
                    ps_w = psw_pool.tile([P, W], f32, tag="psw")
                    for q in range(wq):
                        g = h * wq + q
                        nc.tensor.matmul(
                            ps_w[:, q * 512 : (q + 1) * 512],
                            lhsT=mm_ap(svt_sb[:, m * P : (m + 1) * P]),
                            rhs=mm_ap(vt_sb[:, g * 512 : (g + 1) * 512]),
                            start=True,
                            stop=True,
                        )
                    dst = ob[:, h * W : (h + 1) * W]
                    if h % 2 == 0:
                        nc.scalar.copy(dst, ps_w)
                    else:
                        nc.vector.tensor_copy(dst, ps_w)
                if m % 2 == 0:
                    nc.sync.dma_start(out=out_d[p, m * P : (m + 1) * P, :], in_=ob)
                else:
                    nc.gpsimd.dma_start(out=out_d[p, m * P : (m + 1) * P, :], in_=ob)

    if spill:
        _spill_waits(nc)
    return nc


def _spill_waits(nc, multi_ok=("EventSemaphore",), max_keep=1):
    """Walrus encodes at most one sync-wait on Matmult (embedded weight load)
    and DMACopy; move extra waits onto a preceding same-engine EventSemaphore
    (which supports many waits). The engine sequencer processes instructions
    in order, so a preceding wait is semantically identical."""
    from concourse import mybir

    n_spilled = 0
    for f in nc.m.functions:
        for bb in f.blocks:
            il = bb.instructions
            out = []
            for inst in il:
                si = getattr(inst, "sync_info", None)
                waits = list((si.on_wait if si else None) or [])
                cap = 2 if inst.opcode in multi_ok else max_keep
                if len(waits) > cap:
                    moved, keep = waits[:-max_keep], waits[-max_keep:]
                    for k in range(0, len(moved), 2):
                        es = mybir.InstEventSemaphore(
                            name=f"{inst.name}-wspill{k}",
                            engine=inst.engine,
                            ins=[],
                            outs=[],
                            sync_info=mybir.SyncInfo(
                                on_wait=moved[k : k + 2], on_update=[]
                            ),
                        )
                        out.append(es)
                    inst.sync_info = mybir.SyncInfo(
                        on_wait=keep, on_update=list(si.on_update or [])
                    )
                    n_spilled += 1
                out.append(inst)
            il[:] = out
    return n_spilled


def _import_concourse():
    try:
        import concourse  # noqa: F401
    except ImportError:
        import sys

        for p in ("/opt/trn_rl_repo", "/root/.axon_site/_ro/trn_rl_repo"):
            if p not in sys.path:
                sys.path.insert(0, p)


def _ensure_device_backend():
    """If the process pinned JAX_PLATFORMS to cpu, lift the pin so the
    NeuronCores (axon platform) are reachable for the kernel run."""
    import os

    plats = os.environ.get("JAX_PLATFORMS", "")
    if plats and "axon" not in plats and "neuron" not in plats:
        os.environ["JAX_PLATFORMS"] = ""
        try:
            import jax

            jax.extend.backend.clear_backends()
        except Exception:
            pass


def kernel(x, A, window_size=None):
    _import_concourse()
    _ensure_device_backend()
    from concourse.bass_utils import run_bass_kernel_spmd

    x = np.ascontiguousarray(x, dtype=np.float32)
    A = np.ascontiguousarray(A, dtype=np.float32)
    assert x.shape == (B, T, D) and A.shape == (H, Dh, Dh)

    nc = _COMPILED.get(MM_DTYPE)
    if nc is None:
        nc = _build_nc(mm_dtype_name=MM_DTYPE)
        _COMPILED[MM_DTYPE] = nc

    # x[b, t, h*64:(h+1)*64] per (b,h) pair; pair index bh = b*H + h.
    xv = x.reshape(B, T, H, Dh).transpose(0, 2, 1, 3).reshape(B * H, T, Dh)
    S = (A - np.swapaxes(A, -1, -2)).astype(np.float32)  # replicated with heads
    S_all = np.tile(S, (B, 1, 1))
    ident = np.eye(P, dtype=np.float32)
    in_maps = []
    for c in range(N_CORES):
        sl = slice(c * PAIRS, (c + 1) * PAIRS)
        in_maps.append(
            {
                "x": np.ascontiguousarray(xv[sl]),
                "s": np.ascontiguousarray(S_all[sl]),
                "ident": ident,
            }
        )
    res = run_bass_kernel_spmd(nc, in_maps, list(range(N_CORES)), trace=TRACE)
    global LAST_RESULT
    LAST_RESULT = res
    outs = [res.results[c]["out"] for c in range(N_CORES)]
    full = np.concatenate(outs, axis=0).reshape(B, H, T, T)
    return full


# revision 33
# speedup vs baseline: 1.1572x; 1.1572x over previous
"""Trainium2 Bass kernel for nn_DirectionalWedgeBias.

Computes, per (batch b, head h):
    v      = x[b].reshape(T, H, Dh)[:, h, :]          # [T, Dh]
    v_hat  = v / max(||v||_2, eps)  (row-wise)
    S      = A[h] - A[h]^T                            # [Dh, Dh]
    wedge  = (v_hat @ S) @ v_hat^T                    # [T, T]

Full shapes: x [2, 2048, 1024] f32, A [16, 64, 64] f32 -> out [2, 16, 2048, 2048] f32.

Sharding: 32 independent (b, h) pairs split 4-per-core across 8 NeuronCores
(data + head parallel; the tiny skew-symmetric S is replicated/sliced with the
heads). Host pre-slices x into per-core [4, T, Dh] blocks, forms S = A - A^T,
and re-stacks the per-core [4, T, T] results.

Per-core dataflow (Tile framework):
  - load v [2048, 64] as [128 parts, 16, 64]; row-normalize on DVE
    (square+reduce, ACT sqrt, DVE reciprocal, one broadcast multiply)
  - PE-transpose to vT [64, 2048] (Dh on partitions), f32r-rounded
  - SvT [64, 2048] = matmul(lhsT=S, rhs=vT), float32r (1 cyc/row vs 4 for
    fp32; measured rel err ~2e-4 against the fp32 reference)
  - wedge m-tiles: 4 matmuls (N=512, K=64) per [128, 2048] row block; PSUM
    evacuation alternates ScalarE/VectorE; 1 MiB stores alternate between
    the HWDGE (sync) ring and SWDGE (gpsimd) to overlap DMA issue overheads
  - walrus encodes at most ONE semaphore wait on most instructions (and two
    on EventSemaphore), so `_spill_waits` post-processes the Tile-scheduled
    BIR, hoisting excess waits onto preceding same-engine EventSemaphores
    (sequencers run in order, so this is semantics-preserving)

Cost-model (CoreSim) per-core time: ~141 us; HBM write floor for the
64 MiB/core output is ~187 us at ~358 GB/s per core.
"""

import numpy as np

B = 2
T = 2048
D = 1024
H = 16
Dh = 64
N_CORES = 8
PAIRS = (B * H) // N_CORES  # 4 per core
P = 128  # SBUF partitions

_COMPILED = {}

# test-harness knobs (default off; harness calls kernel() with these untouched)
TRACE = False
MM_DTYPE = "float32r"
LAST_RESULT = None


def _build_nc(pairs=PAIRS, t=T, mm_dtype_name="float32r", spill=True, repeat=1):
    _import_concourse()
    from contextlib import ExitStack

    import concourse.bass as bass
    import concourse.tile as tile
    from concourse import mybir

    f32 = mybir.dt.float32
    mmdt = getattr(mybir.dt, mm_dtype_name)
    nt = t // P  # t-tiles per pair
    ng = t // 512  # 512-wide col groups

    def mm_ap(ap):
        return ap.bitcast(mmdt) if mmdt is not f32 else ap

    nc = bass.Bass()
    x_in = nc.declare_dram_parameter("x", [pairs, t, Dh], f32, isOutput=False)
    s_in = nc.declare_dram_parameter("s", [pairs, Dh, Dh], f32, isOutput=False)
    id_in = nc.declare_dram_parameter("ident", [P, P], f32, isOutput=False)
    out_d = nc.declare_dram_parameter("out", [pairs, t, t], f32, isOutput=True)

    with ExitStack() as ctx:
        tc = ctx.enter_context(tile.TileContext(nc))
        const_pool = ctx.enter_context(tc.tile_pool(name="const", bufs=1))
        stage_pool = ctx.enter_context(tc.tile_pool(name="stage", bufs=2))
        pair_pool = ctx.enter_context(tc.tile_pool(name="pair", bufs=2))
        norm_pool = ctx.enter_context(tc.tile_pool(name="norm", bufs=2))
        psw_pool = ctx.enter_context(tc.tile_pool(name="psw", bufs=3, space="PSUM"))
        pst_pool = ctx.enter_context(tc.tile_pool(name="pst", bufs=2, space="PSUM"))
        out_pool = ctx.enter_context(tc.tile_pool(name="outb", bufs=8))

        # identity: DMA-landed, staged through ACT so matmuls only wait on ACT
        id_dma = const_pool.tile([P, P], f32)
        nc.sync.dma_start(out=id_dma, in_=id_in[:, :])
        identity = const_pool.tile([P, P], f32)
        nc.scalar.copy(identity, id_dma)
        # warmup matmul: absorbs the ACT(identity) wait so the first real
        # transpose only needs its DVE wait
        ps_warm = pst_pool.tile([Dh, 512], f32, tag="pst")
        nc.tensor.matmul(
            ps_warm[:1, :1],
            lhsT=identity[:1, :1],
            rhs=identity[:1, :1],
            start=True,
            stop=True,
        )

        for p in [q for _ in range(repeat) for q in range(pairs)]:
            # ---- S (precomputed skew-symmetric), staged through ACT ----
            s_dma = stage_pool.tile([Dh, Dh], f32, tag="sdma")
            nc.scalar.dma_start(out=s_dma, in_=s_in[p])
            s_sb = pair_pool.tile([Dh, Dh], f32, tag="s")
            nc.scalar.copy(mm_ap(s_sb[:]), s_dma)

            # ---- load v as [128, nt, 64], chunked per 512-row group so the
            #      square/reduce work overlaps the remaining loads ----
            v_sb = pair_pool.tile([P, nt, Dh], f32, tag="v")
            vsq = norm_pool.tile([P, nt, Dh], f32, tag="vsq")
            sumsq = norm_pool.tile([P, nt], f32, tag="ss")
            gn = nt // ng  # n-tiles per group (4)
            for g in range(ng):
                # pair 0 is the pipeline fill: spread its chunk loads over
                # the three idle DMA issue paths so they land concurrently
                if p == 0:
                    ld = (nc.sync, nc.gpsimd, nc.scalar, nc.sync)[g % 4]
                else:
                    ld = nc.scalar
                ld.dma_start(
                    out=v_sb[:, g * gn : (g + 1) * gn, :],
                    in_=x_in[p][g * 512 : (g + 1) * 512, :].rearrange(
                        "(n p) d -> p n d", p=P
                    ),
                )
                nc.vector.tensor_mul(
                    vsq[:, g * gn : (g + 1) * gn, :],
                    v_sb[:, g * gn : (g + 1) * gn, :],
                    v_sb[:, g * gn : (g + 1) * gn, :],
                )
                nc.vector.reduce_sum(
                    sumsq[:, g * gn : (g + 1) * gn],
                    vsq[:, g * gn : (g + 1) * gn, :],
                    axis=mybir.AxisListType.X,
                )
            nrm = norm_pool.tile([P, nt], f32, tag="nrm")
            nc.scalar.activation(nrm, sumsq, mybir.ActivationFunctionType.Sqrt)
            rinv = norm_pool.tile([P, nt], f32, tag="rinv")
            nc.vector.reciprocal(rinv, nrm)

            # per group: normalize (fresh DVE-owned tile), PE-transpose,
            # evacuate, and immediately form that group's SvT slice so the
            # first wedge tiles can start before later groups finish
            v_hat = pair_pool.tile([P, nt, Dh], f32, tag="vhat")
            vt_sb = pair_pool.tile([Dh, t], f32, tag="vt")
            svt_sb = pair_pool.tile([Dh, t], f32, tag="svt")
            for g in range(ng):
                rb = (
                    rinv[:, g * gn : (g + 1) * gn]
                    .unsqueeze(-1)
                    .broadcast_to((P, gn, Dh))
                )
                nc.vector.tensor_mul(
                    v_hat[:, g * gn : (g + 1) * gn, :],
                    v_sb[:, g * gn : (g + 1) * gn, :],
                    rb,
                )
                ps_vt = pst_pool.tile([Dh, 512], f32, tag="pst")
                for j in range(gn):
                    n = g * gn + j
                    nc.tensor.transpose(
                        ps_vt[:, j * P : (j + 1) * P], v_hat[:, n, :], identity
                    )
                nc.vector.tensor_copy(mm_ap(vt_sb[:, g * 512 : (g + 1) * 512]), ps_vt)
                ps_sv = pst_pool.tile([Dh, 512], f32, tag="pst")
                nc.tensor.matmul(
                    ps_sv,
                    lhsT=mm_ap(s_sb[:]),
                    rhs=mm_ap(vt_sb[:, g * 512 : (g + 1) * 512]),
                    start=True,
                    stop=True,
                )
                nc.scalar.copy(mm_ap(svt_sb[:, g * 512 : (g + 1) * 512]), ps_sv)

            # ---- wedge tiles: [128, W] halves into a [128, 2W] out tile;
            #      evacuation alternates ACT/DVE; 1 MiB stores alternate
            #      between the HWDGE (sync) ring and SWDGE (gpsimd) ----
            W = 1024 if ng % 2 == 0 else 512
            wq = W // 512
            halves = t // W
            first_pair = p == 0 and repeat == 1
            for m in range(nt):
                ob = out_pool.tile([P, t], f32, tag="ob")
                for h in range(halves):
                    ps_w = psw_pool.tile([P, W], f32, tag="psw")
                    for q in range(wq):
                        g = h * wq + q
                        nc.tensor.matmul(
                            ps_w[:, q * 512 : (q + 1) * 512],
                            lhsT=mm_ap(svt_sb[:, m * P : (m + 1) * P]),
                            rhs=mm_ap(vt_sb[:, g * 512 : (g + 1) * 512]),
                            start=True,
                            stop=True,
                        )
                    dst = ob[:, h * W : (h + 1) * W]
                    if h % 2 == 0:
                        nc.scalar.copy(dst, ps_w)
                    else:
                        nc.vector.tensor_copy(dst, ps_w)
                    if first_pair and m < 8:
                        # pipeline fill: store each half as soon as copied
                        eng = nc.sync if (m + h) % 2 == 0 else nc.gpsimd
                        eng.dma_start(
                            out=out_d[p, m * P : (m + 1) * P, h * W : (h + 1) * W],
                            in_=dst,
                        )
                if first_pair and m < 8:
                    pass
                elif m % 2 == 0:
                    nc.sync.dma_start(out=out_d[p, m * P : (m + 1) * P, :], in_=ob)
                else:
                    nc.gpsimd.dma_start(out=out_d[p, m * P : (m + 1) * P, :], in_=ob)

    if spill:
        _spill_waits(nc)
    return nc


def _spill_waits(nc, multi_ok=("EventSemaphore",), max_keep=1):
    """Walrus encodes at most one sync-wait on Matmult (embedded weight load)
    and DMACopy; move extra waits onto a preceding same-engine EventSemaphore
    (which supports many waits). The engine sequencer processes instructions
    in order, so a preceding wait is semantically identical."""
    from concourse import mybir

    n_spilled = 0
    for f in nc.m.functions:
        for bb in f.blocks:
            il = bb.instructions
            out = []
            for inst in il:
                si = getattr(inst, "sync_info", None)
                waits = list((si.on_wait if si else None) or [])
                cap = 2 if inst.opcode in multi_ok else max_keep
                if len(waits) > cap:
                    moved, keep = waits[:-max_keep], waits[-max_keep:]
                    for k in range(0, len(moved), 2):
                        es = mybir.InstEventSemaphore(
                            name=f"{inst.name}-wspill{k}",
                            engine=inst.engine,
                            ins=[],
                            outs=[],
                            sync_info=mybir.SyncInfo(
                                on_wait=moved[k : k + 2], on_update=[]
                            ),
                        )
                        out.append(es)
                    inst.sync_info = mybir.SyncInfo(
                        on_wait=keep, on_update=list(si.on_update or [])
                    )
                    n_spilled += 1
                out.append(inst)
            il[:] = out
    return n_spilled


def _import_concourse():
    try:
        import concourse  # noqa: F401
    except ImportError:
        import sys

        for p in ("/opt/trn_rl_repo", "/root/.axon_site/_ro/trn_rl_repo"):
            if p not in sys.path:
                sys.path.insert(0, p)


def _ensure_device_backend():
    """If the process pinned JAX_PLATFORMS to cpu, lift the pin so the
    NeuronCores (axon platform) are reachable for the kernel run."""
    import os

    plats = os.environ.get("JAX_PLATFORMS", "")
    if plats and "axon" not in plats and "neuron" not in plats:
        os.environ["JAX_PLATFORMS"] = ""
        try:
            import jax

            jax.extend.backend.clear_backends()
        except Exception:
            pass


def kernel(x, A, window_size=None):
    _import_concourse()
    _ensure_device_backend()
    from concourse.bass_utils import run_bass_kernel_spmd

    x = np.ascontiguousarray(x, dtype=np.float32)
    A = np.ascontiguousarray(A, dtype=np.float32)
    assert x.shape == (B, T, D) and A.shape == (H, Dh, Dh)

    nc = _COMPILED.get(MM_DTYPE)
    if nc is None:
        nc = _build_nc(mm_dtype_name=MM_DTYPE)
        _COMPILED[MM_DTYPE] = nc

    # x[b, t, h*64:(h+1)*64] per (b,h) pair; pair index bh = b*H + h.
    xv = x.reshape(B, T, H, Dh).transpose(0, 2, 1, 3).reshape(B * H, T, Dh)
    S = (A - np.swapaxes(A, -1, -2)).astype(np.float32)  # replicated with heads
    S_all = np.tile(S, (B, 1, 1))
    ident = np.eye(P, dtype=np.float32)
    in_maps = []
    for c in range(N_CORES):
        sl = slice(c * PAIRS, (c + 1) * PAIRS)
        in_maps.append(
            {
                "x": np.ascontiguousarray(xv[sl]),
                "s": np.ascontiguousarray(S_all[sl]),
                "ident": ident,
            }
        )
    res = run_bass_kernel_spmd(nc, in_maps, list(range(N_CORES)), trace=TRACE)
    global LAST_RESULT
    LAST_RESULT = res
    outs = [res.results[c]["out"] for c in range(N_CORES)]
    full = np.concatenate(outs, axis=0).reshape(B, H, T, T)
    return full


# revision 34
# speedup vs baseline: 1.1815x; 1.0210x over previous
"""Trainium2 Bass kernel for nn_DirectionalWedgeBias.

Computes, per (batch b, head h):
    v      = x[b].reshape(T, H, Dh)[:, h, :]          # [T, Dh]
    v_hat  = v / max(||v||_2, eps)  (row-wise)
    S      = A[h] - A[h]^T                            # [Dh, Dh]
    wedge  = (v_hat @ S) @ v_hat^T                    # [T, T]

Full shapes: x [2, 2048, 1024] f32, A [16, 64, 64] f32 -> out [2, 16, 2048, 2048] f32.

Sharding: 32 independent (b, h) pairs split 4-per-core across 8 NeuronCores
(data + head parallel; the tiny skew-symmetric S is replicated/sliced with the
heads). Host pre-slices x into per-core [4, T, Dh] blocks, forms S = A - A^T,
and re-stacks the per-core [4, T, T] results.

Per-core dataflow (Tile framework):
  - load v [2048, 64] as [128 parts, 16, 64]; row-normalize on DVE
    (square+reduce, ACT sqrt, DVE reciprocal, one broadcast multiply)
  - PE-transpose to vT [64, 2048] (Dh on partitions), f32r-rounded
  - SvT [64, 2048] = matmul(lhsT=S, rhs=vT), float32r (1 cyc/row vs 4 for
    fp32; measured rel err ~2e-4 against the fp32 reference)
  - wedge m-tiles: 4 matmuls (N=512, K=64) per [128, 2048] row block; PSUM
    evacuation alternates ScalarE/VectorE; 1 MiB stores alternate between
    the HWDGE (sync) ring and SWDGE (gpsimd) to overlap DMA issue overheads
  - wedge PSUM pool is 3 slots x [128,1024] (6 banks) + 2 x [64,512] for
    transposes/Sv, so the PE runs up to 3 half-tiles ahead of the copies
  - pipeline fill: pair 0 spreads its x-chunk loads over the three DMA issue
    paths and stores its first half-tiles individually
  - walrus encodes at most ONE semaphore wait on most instructions (and two
    on EventSemaphore), so `_spill_waits` post-processes the Tile-scheduled
    BIR, hoisting excess waits onto preceding same-engine EventSemaphores
    (sequencers run in order, so this is semantics-preserving)

Cost-model (CoreSim) per-core time: ~121.5 us (engine busy: DVE/SP/Pool
~102 us each); the shared-HBM write floor for the 64 MiB/core output is
~187 us at ~358 GB/s per core, so real silicon likely lands at 150-190 us.
"""

import numpy as np

B = 2
T = 2048
D = 1024
H = 16
Dh = 64
N_CORES = 8
PAIRS = (B * H) // N_CORES  # 4 per core
P = 128  # SBUF partitions

_COMPILED = {}

# test-harness knobs (default off; harness calls kernel() with these untouched)
TRACE = False
MM_DTYPE = "float32r"
LAST_RESULT = None


def _build_nc(pairs=PAIRS, t=T, mm_dtype_name="float32r", spill=True, repeat=1):
    _import_concourse()
    from contextlib import ExitStack

    import concourse.bass as bass
    import concourse.tile as tile
    from concourse import mybir

    f32 = mybir.dt.float32
    mmdt = getattr(mybir.dt, mm_dtype_name)
    nt = t // P  # t-tiles per pair
    ng = t // 512  # 512-wide col groups

    def mm_ap(ap):
        return ap.bitcast(mmdt) if mmdt is not f32 else ap

    nc = bass.Bass()
    x_in = nc.declare_dram_parameter("x", [pairs, t, Dh], f32, isOutput=False)
    s_in = nc.declare_dram_parameter("s", [pairs, Dh, Dh], f32, isOutput=False)
    id_in = nc.declare_dram_parameter("ident", [P, P], f32, isOutput=False)
    out_d = nc.declare_dram_parameter("out", [pairs, t, t], f32, isOutput=True)

    with ExitStack() as ctx:
        tc = ctx.enter_context(tile.TileContext(nc))
        const_pool = ctx.enter_context(tc.tile_pool(name="const", bufs=1))
        stage_pool = ctx.enter_context(tc.tile_pool(name="stage", bufs=2))
        pair_pool = ctx.enter_context(tc.tile_pool(name="pair", bufs=2))
        norm_pool = ctx.enter_context(tc.tile_pool(name="norm", bufs=2))
        psw_pool = ctx.enter_context(tc.tile_pool(name="psw", bufs=3, space="PSUM"))
        pst_pool = ctx.enter_context(tc.tile_pool(name="pst", bufs=2, space="PSUM"))
        out_pool = ctx.enter_context(tc.tile_pool(name="outb", bufs=8))

        # identity: DMA-landed, staged through ACT so matmuls only wait on ACT
        id_dma = const_pool.tile([P, P], f32)
        nc.sync.dma_start(out=id_dma, in_=id_in[:, :])
        identity = const_pool.tile([P, P], f32)
        nc.scalar.copy(identity, id_dma)
        # warmup matmul: absorbs the ACT(identity) wait so the first real
        # transpose only needs its DVE wait
        ps_warm = pst_pool.tile([Dh, 512], f32, tag="pst")
        nc.tensor.matmul(
            ps_warm[:1, :1],
            lhsT=identity[:1, :1],
            rhs=identity[:1, :1],
            start=True,
            stop=True,
        )

        for p in [q for _ in range(repeat) for q in range(pairs)]:
            # ---- S (precomputed skew-symmetric), staged through ACT ----
            s_dma = stage_pool.tile([Dh, Dh], f32, tag="sdma")
            nc.scalar.dma_start(out=s_dma, in_=s_in[p])
            s_sb = pair_pool.tile([Dh, Dh], f32, tag="s")
            nc.scalar.copy(mm_ap(s_sb[:]), s_dma)

            # ---- load v as [128, nt, 64], chunked per 512-row group so the
            #      square/reduce work overlaps the remaining loads ----
            v_sb = pair_pool.tile([P, nt, Dh], f32, tag="v")
            vsq = norm_pool.tile([P, nt, Dh], f32, tag="vsq")
            sumsq = norm_pool.tile([P, nt], f32, tag="ss")
            gn = nt // ng  # n-tiles per group (4)
            for g in range(ng):
                # pair 0 is the pipeline fill: spread its chunk loads over
                # the three idle DMA issue paths so they land concurrently
                if p == 0:
                    ld = (nc.sync, nc.gpsimd, nc.scalar, nc.sync)[g % 4]
                else:
                    ld = nc.scalar
                ld.dma_start(
                    out=v_sb[:, g * gn : (g + 1) * gn, :],
                    in_=x_in[p][g * 512 : (g + 1) * 512, :].rearrange(
                        "(n p) d -> p n d", p=P
                    ),
                )
                nc.vector.tensor_mul(
                    vsq[:, g * gn : (g + 1) * gn, :],
                    v_sb[:, g * gn : (g + 1) * gn, :],
                    v_sb[:, g * gn : (g + 1) * gn, :],
                )
                nc.vector.reduce_sum(
                    sumsq[:, g * gn : (g + 1) * gn],
                    vsq[:, g * gn : (g + 1) * gn, :],
                    axis=mybir.AxisListType.X,
                )
            nrm = norm_pool.tile([P, nt], f32, tag="nrm")
            nc.scalar.activation(nrm, sumsq, mybir.ActivationFunctionType.Sqrt)
            rinv = norm_pool.tile([P, nt], f32, tag="rinv")
            nc.vector.reciprocal(rinv, nrm)

            # per group: normalize (fresh DVE-owned tile), PE-transpose,
            # evacuate, and immediately form that group's SvT slice so the
            # first wedge tiles can start before later groups finish
            v_hat = pair_pool.tile([P, nt, Dh], f32, tag="vhat")
            vt_sb = pair_pool.tile([Dh, t], f32, tag="vt")
            svt_sb = pair_pool.tile([Dh, t], f32, tag="svt")
            for g in range(ng):
                rb = (
                    rinv[:, g * gn : (g + 1) * gn]
                    .unsqueeze(-1)
                    .broadcast_to((P, gn, Dh))
                )
                nc.vector.tensor_mul(
                    v_hat[:, g * gn : (g + 1) * gn, :],
                    v_sb[:, g * gn : (g + 1) * gn, :],
                    rb,
                )
                ps_vt = pst_pool.tile([Dh, 512], f32, tag="pst")
                for j in range(gn):
                    n = g * gn + j
                    nc.tensor.transpose(
                        ps_vt[:, j * P : (j + 1) * P], v_hat[:, n, :], identity
                    )
                nc.vector.tensor_copy(mm_ap(vt_sb[:, g * 512 : (g + 1) * 512]), ps_vt)
                ps_sv = pst_pool.tile([Dh, 512], f32, tag="pst")
                nc.tensor.matmul(
                    ps_sv,
                    lhsT=mm_ap(s_sb[:]),
                    rhs=mm_ap(vt_sb[:, g * 512 : (g + 1) * 512]),
                    start=True,
                    stop=True,
                )
                nc.scalar.copy(mm_ap(svt_sb[:, g * 512 : (g + 1) * 512]), ps_sv)

            # ---- wedge tiles: [128, W] halves into a [128, 2W] out tile;
            #      evacuation alternates ACT/DVE; 1 MiB stores alternate
            #      between the HWDGE (sync) ring and SWDGE (gpsimd) ----
            W = 1024 if ng % 2 == 0 else 512
            wq = W // 512
            halves = t // W
            first_pair = p == 0 and repeat == 1
            for m in range(nt):
                ob = out_pool.tile([P, t], f32, tag="ob")
                for h in range(halves):
                    ps_w = psw_pool.tile([P, W], f32, tag="psw")
                    for q in range(wq):
                        g = h * wq + q
                        nc.tensor.matmul(
                            ps_w[:, q * 512 : (q + 1) * 512],
                            lhsT=mm_ap(svt_sb[:, m * P : (m + 1) * P]),
                            rhs=mm_ap(vt_sb[:, g * 512 : (g + 1) * 512]),
                            start=True,
                            stop=True,
                        )
                    dst = ob[:, h * W : (h + 1) * W]
                    if h % 2 == 0:
                        nc.scalar.copy(dst, ps_w)
                    else:
                        nc.vector.tensor_copy(dst, ps_w)
                    if first_pair and m < 8:
                        # pipeline fill: store each half as soon as copied
                        eng = nc.sync if (m + h) % 2 == 0 else nc.gpsimd
                        eng.dma_start(
                            out=out_d[p, m * P : (m + 1) * P, h * W : (h + 1) * W],
                            in_=dst,
                        )
                if first_pair and m < 8:
                    pass
                elif m % 2 == 0:
                    nc.sync.dma_start(out=out_d[p, m * P : (m + 1) * P, :], in_=ob)
                else:
                    nc.gpsimd.dma_start(out=out_d[p, m * P : (m + 1) * P, :], in_=ob)

    if spill:
        _spill_waits(nc)
    return nc


def _spill_waits(nc, multi_ok=("EventSemaphore",), max_keep=1):
    """Walrus encodes at most one sync-wait on Matmult (embedded weight load)
    and DMACopy; move extra waits onto a preceding same-engine EventSemaphore
    (which supports many waits). The engine sequencer processes instructions
    in order, so a preceding wait is semantically identical."""
    from concourse import mybir

    n_spilled = 0
    for f in nc.m.functions:
        for bb in f.blocks:
            il = bb.instructions
            out = []
            for inst in il:
                si = getattr(inst, "sync_info", None)
                waits = list((si.on_wait if si else None) or [])
                cap = 2 if inst.opcode in multi_ok else max_keep
                if len(waits) > cap:
                    moved, keep = waits[:-max_keep], waits[-max_keep:]
                    for k in range(0, len(moved), 2):
                        es = mybir.InstEventSemaphore(
                            name=f"{inst.name}-wspill{k}",
                            engine=inst.engine,
                            ins=[],
                            outs=[],
                            sync_info=mybir.SyncInfo(
                                on_wait=moved[k : k + 2], on_update=[]
                            ),
                        )
                        out.append(es)
                    inst.sync_info = mybir.SyncInfo(
                        on_wait=keep, on_update=list(si.on_update or [])
                    )
                    n_spilled += 1
                out.append(inst)
            il[:] = out
    return n_spilled


def _import_concourse():
    try:
        import concourse  # noqa: F401
    except ImportError:
        import sys

        for p in ("/opt/trn_rl_repo", "/root/.axon_site/_ro/trn_rl_repo"):
            if p not in sys.path:
                sys.path.insert(0, p)


def _ensure_device_backend():
    """If the process pinned JAX_PLATFORMS to cpu, lift the pin so the
    NeuronCores (axon platform) are reachable for the kernel run."""
    import os

    plats = os.environ.get("JAX_PLATFORMS", "")
    if plats and "axon" not in plats and "neuron" not in plats:
        os.environ["JAX_PLATFORMS"] = ""
        try:
            import jax

            jax.extend.backend.clear_backends()
        except Exception:
            pass


def kernel(x, A, window_size=None):
    _import_concourse()
    _ensure_device_backend()
    from concourse.bass_utils import run_bass_kernel_spmd

    x = np.ascontiguousarray(x, dtype=np.float32)
    A = np.ascontiguousarray(A, dtype=np.float32)
    assert x.shape == (B, T, D) and A.shape == (H, Dh, Dh)

    nc = _COMPILED.get(MM_DTYPE)
    if nc is None:
        nc = _build_nc(mm_dtype_name=MM_DTYPE)
        _COMPILED[MM_DTYPE] = nc

    # x[b, t, h*64:(h+1)*64] per (b,h) pair; pair index bh = b*H + h.
    xv = x.reshape(B, T, H, Dh).transpose(0, 2, 1, 3).reshape(B * H, T, Dh)
    S = (A - np.swapaxes(A, -1, -2)).astype(np.float32)  # replicated with heads
    S_all = np.tile(S, (B, 1, 1))
    ident = np.eye(P, dtype=np.float32)
    in_maps = []
    for c in range(N_CORES):
        sl = slice(c * PAIRS, (c + 1) * PAIRS)
        in_maps.append(
            {
                "x": np.ascontiguousarray(xv[sl]),
                "s": np.ascontiguousarray(S_all[sl]),
                "ident": ident,
            }
        )
    res = run_bass_kernel_spmd(nc, in_maps, list(range(N_CORES)), trace=TRACE)
    global LAST_RESULT
    LAST_RESULT = res
    outs = [res.results[c]["out"] for c in range(N_CORES)]
    full = np.concatenate(outs, axis=0).reshape(B, H, T, T)
    return full
